# revision 1
# baseline (speedup 1.0000x reference)
"""Two-layer GCN (PyG-style gcn_norm with self-loops) on 8 TRN2 NeuronCores.

Self-contained: takes full inputs, shards internally, runs one SPMD Bass
kernel across cores 0-7, returns the full [N, 1] output.

Layout strategy (dst-sharded message passing):
  - nodes sharded by destination across 8 cores; per-core nodes relabeled
    by degree-class so segmented sums have affine access patterns
  - per layer: node matmul/scale -> fp16 gather-table row
    [64 feats | 64 zeros] (256 B) -> AllGather -> per-edge gather with
    dma_gather(transpose=True) in 4 interleaved phases (phase =
    table_row % 4, elem_step = 4 rows keeps int16 indices < 32768) ->
    feature-major tensor-tensor tree segmented sums on DVE -> post ops.
  - normalization trick: agg[d] = dinv[d] * sum_e dinv[src]*h[src]; all
    scaling is per node, never per edge.
"""
import sys

sys.path.insert(0, "/opt/trn_rl_repo")

import numpy as np
import concourse.bacc as bacc
import concourse.mybir as mybir
import concourse.tile as tile
from concourse.masks import make_identity
from concourse.bass_utils import run_bass_kernel_spmd

C = 8                      # cores
P4 = 4                     # gather phases (table row mod 4)
SLICE_MAX = 8192           # max slots per gather call
LADDER = [1, 2, 3, 4, 5, 6, 8, 10, 12, 14, 16, 20, 24, 28, 32, 40, 48, 64,
          96, 128, 192, 256, 384, 512, 1024, 2048, 4096]

fp16 = mybir.dt.float16
f32 = mybir.dt.float32
i16 = mybir.dt.int16

_CACHE = {}


# --------------------------------------------------------------------------
# host-side layout planning
# --------------------------------------------------------------------------
def _plan(edge_index, n_nodes):
    """Single-pass layout: chunk(e) = src % 4 (id-based), and the per-core
    relabeling preserves id mod 4 (quad-interleaved class groups), so table
    rows satisfy row % 4 == id % 4 and classes never need re-derivation."""
    N = n_nodes
    nsh = N // C
    src = np.concatenate([edge_index[0], np.arange(N)]).astype(np.int64)
    dst = np.concatenate([edge_index[1], np.arange(N)]).astype(np.int64)
    deg = np.bincount(dst, minlength=N).astype(np.int64)

    chunk = src % P4
    key4 = dst * P4 + chunk
    cnt4 = np.bincount(key4, minlength=P4 * N).reshape(N, P4)
    cmax = cnt4.max(1)
    topk = int(cmax.max())
    ladder = np.array([k for k in LADDER if k <= 2 * topk + 8] or [8])
    if ladder[-1] < topk:
        ladder = np.append(ladder, topk)
    node_cls = np.searchsorted(ladder, cmax)
    ncls = len(ladder)

    # per (core, class, residue) counts -> common quad profile
    res4 = np.arange(N) % P4
    core_id = np.arange(N) // nsh
    quads = np.zeros((C, ncls), np.int64)
    for c in range(C):
        m = core_id == c
        cnt_cr = np.bincount(node_cls[m] * P4 + res4[m],
                             minlength=ncls * P4).reshape(ncls, P4)
        quads[c] = cnt_cr.max(1)
    common_q = quads.max(0)          # quads per class (common profile)

    # align group sizes: n_g = 4*q_g must make n_g*K_g % 128 == 0 and
    # n_g % 4 == 0 (quads)
    common = np.zeros(ncls, np.int64)
    for g in range(ncls):
        if common_q[g] == 0:
            continue
        K = int(ladder[g])
        atom_d = 128 // int(np.gcd(K, 128))       # dsts per 128-slot atom
        align = int(np.lcm(atom_d, P4))
        common[g] = -(-common_q[g] * P4 // align) * align
    total = int(common.sum())
    nodecap = -(-(total + 4) // 128) * 128

    gstart = np.zeros(ncls + 1, np.int64)
    gstart[1:] = np.cumsum(common)
    loc_of = np.full(N, -1, np.int64)
    row_of = np.full(N, -1, np.int64)
    ids_by_core = []
    allv = np.arange(N)
    for c in range(C):
        ids = np.full(nodecap, -1, np.int64)
        for g in range(ncls):
            if common[g] == 0:
                continue
            for r in range(P4):
                sel = allv[(core_id == c) & (node_cls == g) & (res4 == r)]
                pos = gstart[g] + P4 * np.arange(len(sel)) + r
                loc_of[sel] = pos
                row_of[sel] = c * nodecap + pos
                ids[pos] = sel
        ids_by_core.append(ids)
    assert (loc_of[allv] >= 0).all()
    # check residue preservation
    # (row % 4 == id % 4 since nodecap % 4 == 0 and pos % 4 == id % 4)

    # slot base per local id
    Kvec = np.zeros(nodecap, np.int64)
    for g in range(ncls):
        Kvec[gstart[g]:gstart[g + 1]] = ladder[g]
    Kvec[total:] = 0
    slotbase = np.zeros(nodecap + 1, np.int64)
    slotbase[1:] = np.cumsum(Kvec)
    S = int(slotbase[total])

    # slice plan
    slices = []
    cur_lo, cur, cur_parts = 0, 0, []
    for g in range(ncls):
        if common[g] == 0:
            continue
        K = int(ladder[g])
        lcm = int(np.lcm(K, 128))
        n_atoms = int(common[g]) * K // lcm
        a = 0
        while a < n_atoms:
            room = (SLICE_MAX - cur) // lcm
            take = min(n_atoms - a, room)
            if take == 0:
                slices.append((cur_lo, cur_lo + cur, cur_parts))
                cur_lo += cur
                cur, cur_parts = 0, []
                continue
            nd = take * lcm // K
            dst_lo = int(gstart[g]) + a * (lcm // K)
            cur_parts.append((K, nd, cur, dst_lo))
            cur += take * lcm
            a += take
    if cur:
        slices.append((cur_lo, cur_lo + cur, cur_parts))
    assert sum(hi - lo for lo, hi, _ in slices) == S

    # per-core idx arrays
    dst_core = core_id[dst]
    dst_loc = loc_of[dst]
    srow = row_of[src]
    idx_arrays, deg_rows, deg_tiles = [], [], []
    for c in range(C):
        m = dst_core == c
        e_loc = dst_loc[m]
        e_row = srow[m]
        e_ch = chunk[m]
        o = np.lexsort((e_row, e_loc, e_ch))
        e_loc, e_row, e_ch = e_loc[o], e_row[o], e_ch[o]

        zr = {}
        for t in range(4):
            l = nodecap - 4 + t
            zr[(c * nodecap + l) % P4] = (c * nodecap + l) // P4

        arr = np.full((P4, S), -1, np.int64)
        for k in range(P4):
            mk = e_ch == k
            lk = e_loc[mk]
            rk = e_row[mk]
            _, start_idx, cnts = np.unique(lk, return_index=True,
                                           return_counts=True)
            posin = np.arange(len(lk)) - np.repeat(start_idx, cnts)
            assert (posin < Kvec[lk]).all()
            arr[k, slotbase[lk] + posin] = rk // P4
            neg = arr[k] < 0
            arr[k, neg] = zr[k]
        wr = np.empty((P4, 16, S // 16), np.int16)
        for k in range(P4):
            for lo, hi, _ in slices:
                seg = arr[k, lo:hi].reshape(-1, 16).T
                wr[k, :, lo // 16: hi // 16] = seg.astype(np.int16)
        idx_arrays.append(np.tile(wr, (1, 8, 1)))

        dr = np.full(nodecap, 1e-24, np.float64)
        ids = ids_by_core[c]
        real = ids >= 0
        dr[real] = 1.0 / deg[ids[real]]
        deg_rows.append(dr.astype(np.float16)[None, :])
        deg_tiles.append(dr.astype(np.float32).reshape(-1, 128).T.copy())

    return dict(
        N=N, nodecap=nodecap, total=total, S=S, slices=slices,
        ids_by_core=ids_by_core, idx_arrays=idx_arrays,
        deg_rows=deg_rows, deg_tiles=deg_tiles,
    )


# --------------------------------------------------------------------------
# device kernel builder (SPMD: shapes must not depend on per-core data)
# --------------------------------------------------------------------------
# --------------------------------------------------------------------------
# device kernel builder (SPMD: shapes must not depend on per-core data)
# --------------------------------------------------------------------------
def _build(nodecap, S, slices, fin, fmid, fout, level=99):
    """level: 6=table build only, 9=+AG1 probe, 10=+L1 agg, 11=+stage3+AG2,
    12=+L2 agg, 99=full pipeline."""
    NT = nodecap // 128
    nc = bacc.Bacc("TRN2", target_bir_lowering=False, debug=False,
                   num_devices=C)

    xt_in = nc.declare_dram_parameter("xt", [fin, nodecap], f32,
                                      isOutput=False)
    w1_in = nc.declare_dram_parameter("w1", [fin, fmid], f32, isOutput=False)
    w2_in = nc.declare_dram_parameter("w2", [fmid, fout], f32,
                                      isOutput=False)
    wl_in = nc.declare_dram_parameter("wl", [fout, 1], f32, isOutput=False)
    b1_in = nc.declare_dram_parameter("b1c", [128, 1], f32, isOutput=False)
    b2_in = nc.declare_dram_parameter("b2c", [fout, 1], f32, isOutput=False)
    bl_in = nc.declare_dram_parameter("blc", [1, 1], f32, isOutput=False)
    degr_in = nc.declare_dram_parameter("degrow", [1, nodecap], fp16,
                                        isOutput=False)
    degt_in = nc.declare_dram_parameter("degtile", [128, NT], f32,
                                        isOutput=False)
    idx_in = nc.declare_dram_parameter("idx", [P4, 128, S // 16], i16,
                                       isOutput=False)
    y_out = nc.declare_dram_parameter("y", [1, nodecap], f32, isOutput=True)

    with tile.TileContext(nc) as tc:
        with (
            tc.tile_pool(name="dram", bufs=1, space="DRAM") as dram,
            tc.tile_pool(name="pers", bufs=1) as pers,
            tc.tile_pool(name="work", bufs=3) as work,
            tc.tile_pool(name="msgp", bufs=2) as msgp,
            tc.tile_pool(name="psA", bufs=2, space="PSUM") as psA,
            tc.tile_pool(name="psB", bufs=1, space="PSUM") as psB,
        ):
            def _body():
                # consume params so nothing is pruned at low levels
                if level < 99:
                    for _ap, _shape in ((w2_in, [fmid, fout]), (wl_in, [fout, 1]),
                                        (b2_in, [fout, 1]), (bl_in, [1, 1]),
                                        (b1_in, [128, 1])):
                        _tt = work.tile(_shape, f32, tag="consume")
                        nc.sync.dma_start(
                            out=_tt[:],
                            in_=_ap.ap()[tuple(slice(0, d) for d in _shape)])
                        nc.vector.tensor_scalar_mul(_tt[:], _tt[:], 1.0)

                # ---- persistent small tensors
                w1a = pers.tile([128, fmid], f32)
                w1b = pers.tile([128, fmid], f32)
                nc.sync.dma_start(out=w1a[:], in_=w1_in.ap()[0:128, :])
                nc.sync.dma_start(out=w1b[:], in_=w1_in.ap()[128:256, :])
                dinvt = pers.tile([128, NT], f32)
                dgt = work.tile([128, NT], f32, tag="dgt")
                nc.sync.dma_start(out=dgt[:], in_=degt_in.ap())
                nc.scalar.activation(dinvt[:], dgt[:],
                                     mybir.ActivationFunctionType.Sqrt)
                if level < 99:
                    prf = pers.tile([1, nodecap], f32)
                    nc.vector.memset(prf[:], 7.0)

                # tables
                shard1 = dram.tile([nodecap, 128], fp16)
                shard2 = dram.tile([nodecap, 128], fp16)
                table1 = dram.tile([C * nodecap, 128], fp16)
                table2 = dram.tile([C * nodecap, 128], fp16)
                zt = pers.tile([128, 512], fp16)
                nc.vector.memset(zt[:], 0.0)
                for sh in (shard1, shard2):
                    v = sh[:].rearrange("(a b) c -> b a c", b=128)
                    for t0 in range(0, NT, 4):
                        tw = min(4, NT - t0)
                        nc.sync.dma_start(
                            out=v[:, t0:t0 + tw, :],
                            in_=zt[:, :tw * 128].rearrange(
                                "a (b c) -> a b c", c=128))

                # ---- stage 1: table1 = dinv * (x @ W1)
                for t in range(NT):
                    pt = psA.tile([128, fmid], f32, tag="t1", space="PSUM")
                    for k, wk in ((0, w1a), (1, w1b)):
                        xt_t = work.tile([128, 128], f32, tag="xt")
                        nc.sync.dma_start(
                            out=xt_t[:],
                            in_=xt_in.ap()[k * 128:(k + 1) * 128,
                                           t * 128:(t + 1) * 128])
                        nc.tensor.matmul(pt[:], lhsT=xt_t[:], rhs=wk[:],
                                         start=(k == 0), stop=(k == 1))
                    st = work.tile([128, 128], fp16, tag="st")
                    nc.vector.memset(st[:, fmid:], 0.0)
                    nc.vector.tensor_scalar(
                        out=st[:, 0:fmid], in0=pt[:], scalar1=dinvt[:, t:t + 1],
                        scalar2=None, op0=mybir.AluOpType.mult)
                    nc.sync.dma_start(
                        out=shard1[:][t * 128:(t + 1) * 128, :], in_=st[:])

                if level <= 6:
                    for _t in range(min(5, NT)):
                        pr = work.tile([1, 128], fp16, tag="pr")
                        nc.sync.dma_start(
                            out=pr[:], in_=shard1[:][_t * 128 + 107:
                                                     _t * 128 + 108, :])
                        nc.vector.tensor_copy(
                            out=prf[:, _t * 128:_t * 128 + 128], in_=pr[:])
                    nc.sync.dma_start(out=y_out.ap(), in_=prf[:])
                    return

                nc.gpsimd.collective_compute(
                    "AllGather", mybir.AluOpType.bypass,
                    replica_groups=[list(range(C))],
                    ins=[shard1.opt()], outs=[table1.opt()])

                if level <= 9:
                    for _t in range(min(5, NT)):
                        _r = 3 * nodecap + _t * 128 + 107   # core 3's shard
                        pr = work.tile([1, 128], fp16, tag="pr")
                        nc.sync.dma_start(out=pr[:], in_=table1[:][_r:_r + 1, :])
                        nc.vector.tensor_copy(
                            out=prf[:, _t * 128:_t * 128 + 128], in_=pr[:])
                    nc.sync.dma_start(out=y_out.ap(), in_=prf[:])
                    return

                # ---- aggregation machinery
                acc = pers.tile([128, nodecap], fp16)
                cur = pers.tile([128, nodecap], fp16)
                nc.vector.memset(acc[:], 0.0)
                nc.vector.memset(cur[:], 0.0)

                def agg_layer(table, out_acc, out_cur):
                    tblv = table[:].rearrange("(a b) c -> a (b c)", b=P4)
                    for k in range(P4):
                        tgt = out_acc if k == 0 else out_cur
                        for (lo, hi, parts) in slices:
                            n = hi - lo
                            ix = msgp.tile([128, SLICE_MAX // 16], i16,
                                           tag="ix", bufs=3)
                            nc.sync.dma_start(
                                out=ix[:, :n // 16],
                                in_=idx_in.ap()[k, :, lo // 16:hi // 16])
                            msg = msgp.tile([128, SLICE_MAX], fp16, tag="msg")
                            nc.gpsimd.dma_gather(
                                out_ap=msg[:, :n].rearrange(
                                    "a (b c) -> a b c", b=1),
                                in_ap=tblv[:, k * 128:(k + 1) * 128],
                                idxs_ap=ix[:, :n // 16],
                                num_idxs=n, num_idxs_reg=n,
                                elem_size=128, elem_step=P4 * 128,
                                transpose=True, single_packet=False,
                            )
                            for (K, nd, moff, dlo) in parts:
                                v = msg[:, moff:moff + nd * K].rearrange(
                                    "a (b c) -> a b c", c=K)
                                if K == 1:
                                    nc.vector.tensor_copy(
                                        out=tgt[:, dlo:dlo + nd], in_=v[:, :, 0])
                                    continue
                                L = K
                                while L > 1:
                                    h = L // 2
                                    if L % 2 == 1:
                                        nc.vector.tensor_tensor(
                                            out=v[:, :, 0:1], in0=v[:, :, 0:1],
                                            in1=v[:, :, 2 * h:2 * h + 1],
                                            op=mybir.AluOpType.add)
                                    if h == 1:
                                        nc.vector.tensor_tensor(
                                            out=tgt[:, dlo:dlo + nd],
                                            in0=v[:, :, 0], in1=v[:, :, 1],
                                            op=mybir.AluOpType.add)
                                    else:
                                        nc.vector.tensor_tensor(
                                            out=v[:, :, 0:h], in0=v[:, :, 0:h],
                                            in1=v[:, :, h:2 * h],
                                            op=mybir.AluOpType.add)
                                    L = h
                        if k > 0:
                            for off in range(0, nodecap, 4096):
                                w = min(4096, nodecap - off)
                                nc.vector.tensor_tensor(
                                    out=out_acc[:, off:off + w],
                                    in0=out_acc[:, off:off + w],
                                    in1=out_cur[:, off:off + w],
                                    op=mybir.AluOpType.add)

                # ---- stage 2: layer-1 aggregation
                agg_layer(table1, acc, cur)

                if level <= 10:
                    accf = work.tile([1, nodecap], f32, tag="accf")
                    nc.vector.tensor_copy(out=accf[:], in_=acc[0:1, :])
                    nc.sync.dma_start(out=y_out.ap(), in_=accf[:])
                    return

                # ---- stage 3: h1t = dinv*relu(dinv*s1+b1) -> shard2 rows
                ones_sb = pers.tile([1, 128], fp16)
                nc.vector.memset(ones_sb[:], 1.0)
                ident16 = pers.tile([128, 128], fp16)
                make_identity(nc, ident16[:])
                b1_sb = pers.tile([128, 1], f32)
                nc.sync.dma_start(out=b1_sb[:], in_=b1_in.ap())
                dinvr = pers.tile([1, nodecap], fp16)
                dgs = work.tile([1, nodecap], fp16, tag="dgs", bufs=1)
                nc.sync.dma_start(out=dgs[:], in_=degr_in.ap())
                nc.scalar.activation(dinvr[:], dgs[:],
                                     mybir.ActivationFunctionType.Sqrt)

                def bcast_dinv(sl, w):
                    t = psB.tile([128, 512], f32, tag="bc", space="PSUM")
                    nc.tensor.matmul(t[:, :w], lhsT=ones_sb[:],
                                     rhs=dinvr[:, sl:sl + w],
                                     start=True, stop=True)
                    return t

                for sl in range(0, nodecap, 512):
                    w = min(512, nodecap - sl)
                    bc = bcast_dinv(sl, w)
                    z = work.tile([128, 512], f32, tag="z")
                    nc.vector.tensor_tensor(out=z[:, :w], in0=acc[:, sl:sl + w],
                                            in1=bc[:, :w],
                                            op=mybir.AluOpType.mult)
                    h1 = work.tile([128, 512], f32, tag="h1")
                    nc.scalar.activation(h1[:, :w], z[:, :w],
                                         mybir.ActivationFunctionType.Relu,
                                         bias=b1_sb[:, 0:1])
                    nc.vector.tensor_tensor(out=acc[:, sl:sl + w],
                                            in0=h1[:, :w], in1=bc[:, :w],
                                            op=mybir.AluOpType.mult)
                for t in range(NT):
                    ptr = psA.tile([128, 128], fp16, tag="tr", space="PSUM")
                    nc.tensor.transpose(ptr[:], acc[:, t * 128:(t + 1) * 128],
                                        ident16[:])
                    rw = work.tile([128, 128], fp16, tag="rw")
                    nc.vector.tensor_copy(out=rw[:], in_=ptr[:])
                    nc.sync.dma_start(out=shard2[:][t * 128:(t + 1) * 128, :],
                                      in_=rw[:])

                nc.gpsimd.collective_compute(
                    "AllGather", mybir.AluOpType.bypass,
                    replica_groups=[list(range(C))],
                    ins=[shard2.opt()], outs=[table2.opt()])

                if level <= 11:
                    for _t in range(min(5, NT)):
                        _r = 3 * nodecap + _t * 128 + 107
                        pr = work.tile([1, 128], fp16, tag="pr")
                        nc.sync.dma_start(out=pr[:], in_=table2[:][_r:_r + 1, :])
                        nc.vector.tensor_copy(
                            out=prf[:, _t * 128:_t * 128 + 128], in_=pr[:])
                    nc.sync.dma_start(out=y_out.ap(), in_=prf[:])
                    return

                # ---- stage 4: layer-2 aggregation
                agg_layer(table2, acc, cur)

                if level <= 12:
                    accf = work.tile([1, nodecap], f32, tag="accf")
                    nc.vector.tensor_copy(out=accf[:], in_=acc[0:1, :])
                    nc.sync.dma_start(out=y_out.ap(), in_=accf[:])
                    return

                # ---- stage 5
                w2_sb = pers.tile([fmid, fout], f32)
                nc.sync.dma_start(out=w2_sb[:], in_=w2_in.ap())
                wl_sb = pers.tile([fout, 1], f32)
                nc.sync.dma_start(out=wl_sb[:], in_=wl_in.ap())
                b2_sb = pers.tile([fout, 1], f32)
                nc.sync.dma_start(out=b2_sb[:], in_=b2_in.ap())
                bl_sb = pers.tile([1, 1], f32)
                nc.sync.dma_start(out=bl_sb[:], in_=bl_in.ap())
                for sl in range(0, nodecap, 512):
                    w = min(512, nodecap - sl)
                    bc = bcast_dinv(sl, w)
                    s2 = work.tile([128, 512], f32, tag="s2")
                    nc.vector.tensor_tensor(out=s2[:, :w], in0=acc[:, sl:sl + w],
                                            in1=bc[:, :w],
                                            op=mybir.AluOpType.mult)
                    ph = psA.tile([fout, 512], f32, tag="h2", space="PSUM")
                    nc.tensor.matmul(ph[:, :w], lhsT=w2_sb[:],
                                     rhs=s2[0:fmid, :w], start=True, stop=True)
                    h2 = work.tile([fout, 512], f32, tag="h2s")
                    nc.scalar.activation(h2[:, :w], ph[:, :w],
                                         mybir.ActivationFunctionType.Relu,
                                         bias=b2_sb[:, 0:1])
                    py = psB.tile([1, 512], f32, tag="y", space="PSUM")
                    nc.tensor.matmul(py[:, :w], lhsT=wl_sb[:], rhs=h2[:, :w],
                                     start=True, stop=True)
                    yw = work.tile([1, 512], f32, tag="yw")
                    nc.vector.tensor_scalar(
                        out=yw[:, :w], in0=py[:, :w], scalar1=bl_sb[:, 0:1],
                        scalar2=None, op0=mybir.AluOpType.add)
                    nc.sync.dma_start(out=y_out.ap()[:, sl:sl + w],
                                      in_=yw[:, :w])


            _body()
    nc.compile()
    return nc


# --------------------------------------------------------------------------
# public entry
# --------------------------------------------------------------------------
def kernel(x, edge_index, W1, b1, W2, b2, Wlin, blin):
    x = np.asarray(x, np.float32)
    edge_index = np.asarray(edge_index)
    N, fin = x.shape
    fmid = W1.shape[1]
    fout = W2.shape[1]

    ck = ("plan", edge_index.shape[1], N,
          int(edge_index[:, :100].sum()), int(edge_index[:, -100:].sum()))
    if ck in _CACHE:
        plan = _CACHE[ck]
    else:
        plan = _plan(edge_index, N)
        _CACHE[ck] = plan

    bk = ("nc", plan["nodecap"], plan["S"],
          tuple(tuple(s[:2]) for s in plan["slices"]), fin, fmid, fout)
    if bk in _CACHE:
        nc = _CACHE[bk]
    else:
        nc = _build(plan["nodecap"], plan["S"], plan["slices"],
                    fin, fmid, fout)
        _CACHE[bk] = nc

    nodecap = plan["nodecap"]
    b1c = np.zeros((128, 1), np.float32)
    b1c[:fmid, 0] = np.asarray(b1, np.float32)
    in_maps = []
    for c in range(C):
        ids = plan["ids_by_core"][c]
        xt = np.zeros((fin, nodecap), np.float32)
        real = ids >= 0
        xt[:, real] = x[ids[real]].T
        in_maps.append({
            "xt": xt,
            "w1": np.asarray(W1, np.float32),
            "w2": np.asarray(W2, np.float32),
            "wl": np.asarray(Wlin, np.float32),
            "b1c": b1c,
            "b2c": np.asarray(b2, np.float32).reshape(fout, 1),
            "blc": np.asarray(blin, np.float32).reshape(1, 1),
            "degrow": plan["deg_rows"][c],
            "degtile": plan["deg_tiles"][c],
            "idx": plan["idx_arrays"][c],
        })

    res = run_bass_kernel_spmd(nc, in_maps, list(range(C)))

    y = np.empty((N, 1), np.float32)
    for c in range(C):
        ids = plan["ids_by_core"][c]
        real = ids >= 0
        y[ids[real], 0] = res.results[c]["y"][0, real]
    return y


def timed_run(x, edge_index, W1, b1, W2, b2, Wlin, blin):
    """Run once with NTFF tracing and return HW exec time in ns."""
    # trigger plan/build/cache via a normal call path but with trace
    global _LAST_TRACE
    import kernel as _self  # noqa
    x = np.asarray(x, np.float32)
    edge_index = np.asarray(edge_index)
    N, fin = x.shape
    fmid = W1.shape[1]
    fout = W2.shape[1]
    ck = ("plan", edge_index.shape[1], N,
          int(edge_index[:, :100].sum()), int(edge_index[:, -100:].sum()))
    plan = _CACHE.get(ck) or _plan(edge_index, N)
    _CACHE[ck] = plan
    bk = ("nc", plan["nodecap"], plan["S"],
          tuple(tuple(s[:2]) for s in plan["slices"]), fin, fmid, fout)
    nc = _CACHE.get(bk) or _build(plan["nodecap"], plan["S"],
                                  plan["slices"], fin, fmid, fout)
    _CACHE[bk] = nc
    nodecap = plan["nodecap"]
    b1c = np.zeros((128, 1), np.float32)
    b1c[:fmid, 0] = np.asarray(b1, np.float32)
    in_maps = []
    for c in range(C):
        ids = plan["ids_by_core"][c]
        xt = np.zeros((fin, nodecap), np.float32)
        real = ids >= 0
        xt[:, real] = x[ids[real]].T
        in_maps.append({
            "xt": xt, "w1": np.asarray(W1, np.float32),
            "w2": np.asarray(W2, np.float32),
            "wl": np.asarray(Wlin, np.float32), "b1c": b1c,
            "b2c": np.asarray(b2, np.float32).reshape(fout, 1),
            "blc": np.asarray(blin, np.float32).reshape(1, 1),
            "degrow": plan["deg_rows"][c], "degtile": plan["deg_tiles"][c],
            "idx": plan["idx_arrays"][c],
        })
    res = run_bass_kernel_spmd(nc, in_maps, list(range(C)), trace=True)
    _LAST_TRACE = res
    return res.exec_time_ns

